# revision 18
# baseline (speedup 1.0000x reference)
"""DIEN-style attention-GRU kernel for 8 trn2 NeuronCores.

Sharding: data-parallel over batch (1024 -> 128 per core), weights
replicated, the time scan stays local per shard.

v3: transposed layout [feat, batch] everywhere, bf16 compute with fp32 PSUM
accumulation.  The recurrent update h' = s1 + t2 is *not* materialized on
the critical path: by linearity Wh.h' = Wh.s1 + Wh.t2, so the next step's
r-gate matmul accumulates the t2-stream early (during tanh) and only the
s1-stream (128 cols) remains serial.  Biases ride in ACT bias slots, a PE
outer-product (bg), and fused scalar_tensor_tensor ops.  Attention weights
are broadcast across partitions with one PE outer-product per 4-step chunk.
"""

import sys

sys.path.insert(0, "/opt/trn_rl_repo")

import numpy as np
import ml_dtypes

import concourse.bacc as bacc
import concourse.mybir as mybir
from concourse.tile import TileContext
from concourse.tile_rust import add_dep_helper
from concourse.bass_utils import run_bass_kernel_spmd

B, T, IN, H = 1024, 200, 128, 128
NCORES = 8
BS = B // NCORES  # 128 batches per core

F32 = mybir.dt.float32
BF16 = mybir.dt.bfloat16
AF = mybir.ActivationFunctionType
ALU = mybir.AluOpType

PG = 8    # phase-1 timesteps per chunk (2 PSUM banks of aw per chunk)
XLA = 1   # scan x-side lookahead (steps)
NDMA = 8  # big input DMAs per tensor

# wcat block indices
(W_AW, W_HU, W_HR, W_HG, W_XU, W_XR, W_XG, W_L2H, W_L2T, W_ID, W_ONE,
 W_NHR, W_NHG) = range(13)
NBLK = 13


def build_nc(t_steps=T, num_devices=NCORES):
    nc = bacc.Bacc("TRN2", target_bir_lowering=False, debug=False,
                   num_devices=num_devices)
    NPG = t_steps // PG
    NC4 = t_steps // 4
    assert t_steps % PG == 0 and t_steps % 4 == 0

    tgtT = nc.dram_tensor("tgtT", [IN, t_steps, BS], BF16, kind="ExternalInput")
    histT = nc.dram_tensor("histT", [H, t_steps, BS], BF16,
                           kind="ExternalInput")
    wcat = nc.dram_tensor("wcat", [128, NBLK * 128], BF16, kind="ExternalInput")
    bcols = nc.dram_tensor("bcols", [H, 4], F32, kind="ExternalInput")
    brows = nc.dram_tensor("brows", [1, 3 * 128], BF16, kind="ExternalInput")
    out_d = nc.dram_tensor("out", [BS, H], F32, kind="ExternalOutput")

    with TileContext(nc) as tc:
        with (
            tc.tile_pool(name="const", bufs=1) as constp,
            tc.tile_pool(name="big", bufs=1) as bigp,
            tc.tile_pool(name="p1sb", bufs=3) as p1sb,
            tc.tile_pool(name="att", bufs=1) as attp,
            tc.tile_pool(name="scan", bufs=3) as scanp,
            tc.tile_pool(name="state", bufs=3) as statep,
        ):
            # ---- first input slice, then constants, then the rest ----
            tgt_all = bigp.tile([128, t_steps, BS], BF16, tag="tgt_all")
            hist_all = bigp.tile([128, t_steps, BS], BF16, tag="hist_all")
            TSL = t_steps // NDMA
            for d in range(1):
                sl = slice(d * TSL, (d + 1) * TSL)
                nc.sync.dma_start(tgt_all[:, sl, :], tgtT[:, sl, :])
                nc.scalar.dma_start(hist_all[:, sl, :], histT[:, sl, :])
            wcat_s = constp.tile([128, NBLK * 128], BF16, tag="wcat")
            nc.sync.dma_start(wcat_s[:], wcat[:, :])
            bcols_s = constp.tile([H, 4], F32, tag="bcols")
            nc.sync.dma_start(bcols_s[:], bcols[:, :])
            brows_s = constp.tile([1, 3 * 128], BF16, tag="brows")
            nc.sync.dma_start(brows_s[:], brows[:, :])

            def blk(i):
                return wcat_s[:, i * 128:(i + 1) * 128]

            wWT_s = blk(W_AW)
            WhuT_s, WhrT_s, WhgT_s = blk(W_HU), blk(W_HR), blk(W_HG)
            WxuT_s, WxrT_s, WxgT_s = blk(W_XU), blk(W_XR), blk(W_XG)
            ln2wh_s, ln2wt_s, ident_s = blk(W_L2H), blk(W_L2T), blk(W_ID)
            nWhrT_s = blk(W_NHR)
            nWhgT_s = blk(W_NHG)
            ones_row_s = wcat_s[0:1, W_ONE * 128:(W_ONE + 1) * 128]
            ones_col_s = wcat_s[:, W_ONE * 128:W_ONE * 128 + 1]
            wb_s = bcols_s[:, 0:1]
            bu_s = bcols_s[:, 1:2]
            br_s = bcols_s[:, 2:3]
            ln2b_s = brows_s[:, 0:H]
            bg_row_s = brows_s[:, 128:128 + H]
            bq_row_s = brows_s[:, 256:256 + H]

            # ---- remaining input DMA slices, interleaved tgt/hist ----
            for d in range(1, NDMA):
                sl = slice(d * TSL, (d + 1) * TSL)
                nc.sync.dma_start(tgt_all[:, sl, :], tgtT[:, sl, :])
                nc.scalar.dma_start(hist_all[:, sl, :], histT[:, sl, :])

            # warm the ACT exp table while DMA streams
            dummy = attp.tile([1, 1], F32, tag="dummy")
            nc.vector.memset(dummy[:], 0.0)
            nc.scalar.activation(dummy[:], dummy[:], AF.Exp)

            attT = attp.tile([100, 2, BS], BF16, tag="attT")

            # ================= phase 1: attention =================
            with (
                tc.tile_pool(name="awps", bufs=2, space="PSUM") as awps,
                tc.tile_pool(name="lgps", bufs=1, space="PSUM") as lgps,
            ):
                logits_ps = lgps.tile([BS, t_steps], F32, tag="logits")
                for g in range(NPG):
                    t0 = g * PG
                    awt = awps.tile([H, PG, BS], F32, tag="aw")
                    hpg = PG // 2
                    for hh in range(2):
                        s2 = slice(t0 + hh * hpg, t0 + (hh + 1) * hpg)
                        nc.tensor.matmul(
                            awt[:, hh * hpg:(hh + 1) * hpg, :].rearrange(
                                "i t b -> i (t b)"),
                            wWT_s,
                            tgt_all[:, s2, :].rearrange("i t b -> i (t b)"),
                            start=True, stop=True)
                    # prod = (aw + W_b) * hist   (W_b fused per-partition)
                    prod = p1sb.tile([H, PG, BS], BF16, tag="prod")
                    nc.vector.scalar_tensor_tensor(
                        prod[:].rearrange("h t b -> h (t b)"),
                        awt[:].rearrange("h t b -> h (t b)"),
                        wb_s,
                        hist_all[:, t0:t0 + PG, :].rearrange("h t b -> h (t b)"),
                        ALU.add, ALU.mult)
                    # logits[:, t] = ones^T . prod_t  (partition reduce on PE)
                    for j in range(PG):
                        nc.tensor.matmul(
                            logits_ps[:, t0 + j:t0 + j + 1],
                            prod[:, j, :],
                            ones_col_s,
                            start=True, stop=True)

                # ---- softmax over time (free dim) ----
                mx = attp.tile([BS, 1], F32, tag="mx")
                nc.vector.tensor_reduce(mx[:], logits_ps[:],
                                        mybir.AxisListType.X, ALU.max)
                negmx = attp.tile([BS, 1], F32, tag="negmx")
                nc.vector.tensor_scalar_mul(negmx[:], mx[:], -1.0)
                exps = attp.tile([BS, t_steps], F32, tag="exps")
                nc.scalar.activation(exps[:], logits_ps[:], AF.Exp,
                                     bias=negmx[:])
                # swap in the sigmoid/tanh table for the scan
                nc.scalar.activation(dummy[:], dummy[:], AF.Sigmoid)
                ssum = attp.tile([BS, 1], F32, tag="ssum")
                nc.vector.tensor_reduce(ssum[:], exps[:], mybir.AxisListType.X,
                                        ALU.add)
                rsum = attp.tile([BS, 1], F32, tag="rsum")
                nc.vector.reciprocal(rsum[:], ssum[:])
                att = attp.tile([BS, t_steps], BF16, tag="attn")
                nc.vector.tensor_scalar_mul(att[:], exps[:], rsum[:])
                # transpose att -> attT rows (PE transpose, two halves)
                for hf in range(2):
                    tps = awps.tile([100, BS], BF16, tag="aw")
                    nc.tensor.transpose(tps[:], att[:, hf * 100:(hf + 1) * 100],
                                        ident_s)
                    nc.vector.tensor_copy(attT[:, hf, :], tps[:])

            # ================= phase 2: the scan =================
            with (
                tc.tile_pool(name="rqps", bufs=2, space="PSUM") as rqps,
                tc.tile_pool(name="mps", bufs=2, space="PSUM") as mps,
                tc.tile_pool(name="ups", bufs=2, space="PSUM") as ups,
                tc.tile_pool(name="abps", bufs=2, space="PSUM") as abps,
            ):
                h_t = statep.tile([H, BS], BF16, tag="h")
                nc.vector.memset(h_t[:], 0.0)

                rq_tiles = {}
                m_tiles = {}
                u_tiles = {}
                abc_tiles = {}
                row4_tiles = {}
                s1_prev = None
                t2_prev = None

                def row4_fill(c):
                    t0 = c * 4
                    row4 = scanp.tile([1, 4, BS], BF16, tag="arow")
                    nc.sync.dma_start(
                        row4[:], attT[t0 % 100:t0 % 100 + 4, t0 // 100, :])
                    row4_tiles[c] = row4

                def abc_fill(c):
                    row4 = row4_tiles.pop(c)
                    ab = abps.tile([128, 4, BS], F32, tag="abc")
                    nc.tensor.matmul(
                        ab[:].rearrange("p t b -> p (t b)"),
                        ones_row_s,
                        row4[:].rearrange("p t b -> p (t b)"),
                        start=True, stop=True)
                    abc_tiles[c] = ab

                def x_fill(t):
                    # x-side projections + bias outer-products for step t.
                    # rq bank [r | q]: readers sig_r then gpre (far apart).
                    # m bank: reader rm only.  u bank: reader sig_u only.
                    ht = hist_all[:, t, :]
                    rqt = rqps.tile([H, 2, BS], F32, tag="rq")
                    mt = mps.tile([H, BS], F32, tag="m")
                    ut = ups.tile([H, BS], F32, tag="u")
                    nc.tensor.matmul(rqt[:, 0, :], WxrT_s, ht,
                                     start=True, stop=False)
                    nc.tensor.matmul(rqt[:, 1, :], WxgT_s, ht,
                                     start=False, stop=False,
                                     skip_group_check=True)
                    nc.tensor.matmul(rqt[:, 1, :], bq_row_s, ones_row_s[:, :BS],
                                     start=False, stop=False,
                                     skip_group_check=True)
                    nc.tensor.matmul(mt[:], bg_row_s, ones_row_s[:, :BS],
                                     start=True, stop=False)
                    nc.tensor.matmul(ut[:], WxuT_s, ht,
                                     start=True, stop=False)
                    rq_tiles[t] = rqt
                    m_tiles[t] = mt
                    u_tiles[t] = ut

                def consume(t, h_cur):
                    nonlocal s1_prev, t2_prev
                    rqt = rq_tiles.pop(t)
                    mt = m_tiles.pop(t)
                    ut = u_tiles.pop(t)
                    ab = abc_tiles[t // 4]
                    if t > 0:
                        # r/g gates: Wh.h = -Wh.t2n + Wh.s1 (t2n-streams land
                        # during the previous tanh; only r_s1 is serial and
                        # it alone gates sig_r's bank)
                        nc.tensor.matmul(rqt[:, 0, :], nWhrT_s, t2_prev[:],
                                         start=False, stop=False,
                                         skip_group_check=True)
                        nc.tensor.matmul(mt[:], nWhgT_s, t2_prev[:],
                                         start=False, stop=False,
                                         skip_group_check=True)
                        nc.tensor.matmul(rqt[:, 0, :], WhrT_s, s1_prev[:],
                                         start=False, stop=True,
                                         skip_group_check=True)
                        nc.tensor.matmul(mt[:], WhgT_s, s1_prev[:],
                                         start=False, stop=True,
                                         skip_group_check=True)
                        nc.tensor.matmul(ut[:], WhuT_s, h_cur[:],
                                         start=False, stop=True,
                                         skip_group_check=True)
                    r = scanp.tile([H, BS], BF16, tag="r")
                    nc.scalar.activation(r[:], rqt[:, 0, :], AF.Sigmoid,
                                         bias=br_s)
                    u = scanp.tile([H, BS], BF16, tag="u")
                    nc.scalar.activation(u[:], ut[:], AF.Sigmoid,
                                         bias=bu_s)
                    # rm = (mg + bg) * r     (bg pre-accumulated in PSUM)
                    rm = scanp.tile([H, BS], BF16, tag="rmv")
                    nc.vector.tensor_tensor(rm[:], mt[:], r[:], ALU.mult)
                    # gpre = (xq + bq) + rm  (q copied to SBUF on ACT in
                    # the previous tanh window -> DVE 2x mode)
                    q_sb = scanp.tile([H, BS], BF16, tag="qsb")
                    nc.scalar.copy(q_sb[:], rqt[:, 1, :])
                    gpre = scanp.tile([H, BS], BF16, tag="gpre")
                    bi_gpre = nc.vector.tensor_tensor(gpre[:], q_sb[:],
                                                      rm[:], ALU.add)
                    g_ = scanp.tile([H, BS], BF16, tag="g")
                    nc.scalar.activation(g_[:], gpre[:], AF.Tanh)
                    # v = a_t * u (DVE, behind gpre);
                    # t2n = (v - 1) * h = -(1-v)h in ONE fused op
                    v = scanp.tile([H, BS], BF16, tag="v")
                    bi_v = nc.vector.tensor_tensor(v[:], u[:], ab[:, t % 4, :],
                                                   ALU.mult)
                    add_dep_helper(bi_v.ins, bi_gpre.ins, sync=False,
                                   reason="scan: v after gpre (DVE order)")
                    t2 = scanp.tile([H, BS], BF16, tag="t2")
                    nc.vector.scalar_tensor_tensor(t2[:], v[:], 1.0, h_cur[:],
                                                   ALU.subtract, ALU.mult)
                    # s1 = g * v  (serial), then h' = s1 - t2n (off-path)
                    s1 = scanp.tile([H, BS], BF16, tag="s1")
                    nc.vector.tensor_tensor(s1[:], g_[:], v[:], ALU.mult)
                    h_new = statep.tile([H, BS], BF16, tag="h")
                    nc.vector.tensor_tensor(h_new[:], s1[:], t2[:],
                                            ALU.subtract)
                    s1_prev, t2_prev = s1, t2
                    return h_new

                row4_fill(0)
                row4_fill(1)
                abc_fill(0)
                for t in range(-XLA, t_steps):
                    if t >= 0 and t % 4 == 2:
                        c = t // 4
                        if c + 2 < NC4:
                            row4_fill(c + 2)
                        if c + 1 < NC4:
                            abc_fill(c + 1)
                    tf = t + XLA
                    if tf < t_steps:
                        x_fill(tf)
                    if t >= 0:
                        h_t = consume(t, h_t)

                # ---- phase 3: out = [h, targets[:,0]] @ ln2_w.T + ln2_b ----
                ops = rqps.tile([BS, H], F32, tag="rq")
                nc.tensor.matmul(ops[:], ones_row_s[:, :BS], ln2b_s,
                                 start=True, stop=False)
                nc.tensor.matmul(ops[:], h_t[:], ln2wh_s,
                                 start=False, stop=False, skip_group_check=True)
                nc.tensor.matmul(ops[:], tgt_all[:, 0, :], ln2wt_s,
                                 start=False, stop=True, skip_group_check=True)
                out_s = scanp.tile([BS, H], F32, tag="out_s")
                nc.vector.tensor_copy(out_s[:], ops[:])
                nc.sync.dma_start(out_d[:, :], out_s[:])

    nc.compile()
    return nc


def make_weight_feeds(inputs, t_steps=T):
    f32 = np.float32
    bf16 = ml_dtypes.bfloat16

    def tb(x):  # transpose to [in, out], fp32 -> bf16
        return np.ascontiguousarray(np.asarray(x, dtype=f32).T).astype(bf16)

    ln2_w = np.asarray(inputs["ln2_w"], f32)
    wblocks = [
        tb(inputs["W_w"]), tb(inputs["hu_w"]), tb(inputs["hr_w"]),
        tb(inputs["hg_w"]), tb(inputs["xu_w"]), tb(inputs["xr_w"]),
        tb(inputs["xg_w"]),
        np.ascontiguousarray(ln2_w[:, :H].T).astype(bf16),
        np.ascontiguousarray(ln2_w[:, H:].T).astype(bf16),
        np.eye(128, dtype=f32).astype(bf16),
        np.ones((128, 128), dtype=f32).astype(bf16),
        tb(-np.asarray(inputs["hr_w"], f32)),
        tb(-np.asarray(inputs["hg_w"], f32)),
    ]
    bcols = np.stack([
        np.asarray(inputs["W_b"], f32),
        np.asarray(inputs["xu_b"], f32) + np.asarray(inputs["hu_b"], f32),
        np.asarray(inputs["xr_b"], f32) + np.asarray(inputs["hr_b"], f32),
        np.asarray(inputs["xg_b"], f32),
    ], axis=1)
    brows = np.zeros((1, 3 * 128), f32)
    brows[0, :H] = np.asarray(inputs["ln2_b"], f32)
    brows[0, 128:128 + H] = np.asarray(inputs["hg_b"], f32)
    brows[0, 256:256 + H] = np.asarray(inputs["xg_b"], f32)
    return {
        "wcat": np.ascontiguousarray(np.concatenate(wblocks, axis=1)),
        "bcols": np.ascontiguousarray(bcols),
        "brows": brows.astype(bf16),
    }


def make_core_feeds(inputs, core, t_steps=T):
    bf16 = ml_dtypes.bfloat16
    sl = slice(core * BS, (core + 1) * BS)
    tgt = np.asarray(inputs["targets"])[sl, :t_steps]
    hist = np.asarray(inputs["history_states"])[sl, :t_steps]
    return {
        # [BS, T, F] -> [F, T, BS]
        "tgtT": np.ascontiguousarray(tgt.transpose(2, 1, 0)).astype(bf16),
        "histT": np.ascontiguousarray(hist.transpose(2, 1, 0)).astype(bf16),
    }


_nc_cache = {}


def _get_nc(t_steps=T):
    if t_steps not in _nc_cache:
        _nc_cache[t_steps] = build_nc(t_steps)
    return _nc_cache[t_steps]


def kernel(**inputs):
    nc = _get_nc(T)
    wf = make_weight_feeds(inputs)
    in_maps = [{**make_core_feeds(inputs, c), **wf} for c in range(NCORES)]
    res = run_bass_kernel_spmd(nc, in_maps, list(range(NCORES)))
    out = np.concatenate([res.results[c]["out"] for c in range(NCORES)], axis=0)
    return out.astype(np.float32)


# revision 19
# speedup vs baseline: 1.0159x; 1.0159x over previous
"""DIEN-style attention-GRU kernel for 8 trn2 NeuronCores.

Sharding: data-parallel over batch (1024 -> 128 per core), weights
replicated, the time scan stays local per shard.

v3: transposed layout [feat, batch] everywhere, bf16 compute with fp32 PSUM
accumulation.  The recurrent update h' = s1 + t2 is *not* materialized on
the critical path: by linearity Wh.h' = Wh.s1 + Wh.t2, so the next step's
r-gate matmul accumulates the t2-stream early (during tanh) and only the
s1-stream (128 cols) remains serial.  Biases ride in ACT bias slots, a PE
outer-product (bg), and fused scalar_tensor_tensor ops.  Attention weights
are broadcast across partitions with one PE outer-product per 4-step chunk.
"""

import sys

sys.path.insert(0, "/opt/trn_rl_repo")

import numpy as np
import ml_dtypes

import concourse.bacc as bacc
import concourse.mybir as mybir
from concourse.tile import TileContext
from concourse.tile_rust import add_dep_helper
from concourse.bass_utils import run_bass_kernel_spmd

B, T, IN, H = 1024, 200, 128, 128
NCORES = 8
BS = B // NCORES  # 128 batches per core

F32 = mybir.dt.float32
BF16 = mybir.dt.bfloat16
AF = mybir.ActivationFunctionType
ALU = mybir.AluOpType

PG = 4    # phase-1 timesteps per chunk (1 PSUM bank per tile)
XLA = 1   # scan x-side lookahead (steps)
NDMA = 8  # big input DMAs per tensor

# wcat block indices
(W_AW, W_HU, W_HR, W_HG, W_XU, W_XR, W_XG, W_L2H, W_L2T, W_ID, W_ONE,
 W_NHR, W_NHG) = range(13)
NBLK = 13


def build_nc(t_steps=T, num_devices=NCORES):
    nc = bacc.Bacc("TRN2", target_bir_lowering=False, debug=False,
                   num_devices=num_devices)
    NPG = t_steps // PG
    NC4 = t_steps // 4
    assert t_steps % PG == 0 and t_steps % 4 == 0

    tgtT = nc.dram_tensor("tgtT", [IN, t_steps, BS], BF16, kind="ExternalInput")
    histT = nc.dram_tensor("histT", [H, t_steps, BS], BF16,
                           kind="ExternalInput")
    wcat = nc.dram_tensor("wcat", [128, NBLK * 128], BF16, kind="ExternalInput")
    bcols = nc.dram_tensor("bcols", [H, 4], F32, kind="ExternalInput")
    brows = nc.dram_tensor("brows", [1, 3 * 128], BF16, kind="ExternalInput")
    out_d = nc.dram_tensor("out", [BS, H], F32, kind="ExternalOutput")

    with TileContext(nc) as tc:
        with (
            tc.tile_pool(name="const", bufs=1) as constp,
            tc.tile_pool(name="big", bufs=1) as bigp,
            tc.tile_pool(name="p1sb", bufs=3) as p1sb,
            tc.tile_pool(name="att", bufs=1) as attp,
            tc.tile_pool(name="scan", bufs=3) as scanp,
            tc.tile_pool(name="state", bufs=3) as statep,
        ):
            # ---- first input slice, then constants, then the rest ----
            tgt_all = bigp.tile([128, t_steps, BS], BF16, tag="tgt_all")
            hist_all = bigp.tile([128, t_steps, BS], BF16, tag="hist_all")
            xq_all = bigp.tile([128, t_steps, BS], BF16, tag="xq_all")
            TSL = t_steps // NDMA
            for d in range(1):
                sl = slice(d * TSL, (d + 1) * TSL)
                nc.sync.dma_start(tgt_all[:, sl, :], tgtT[:, sl, :])
                nc.scalar.dma_start(hist_all[:, sl, :], histT[:, sl, :])
            wcat_s = constp.tile([128, NBLK * 128], BF16, tag="wcat")
            nc.sync.dma_start(wcat_s[:], wcat[:, :])
            bcols_s = constp.tile([H, 4], F32, tag="bcols")
            nc.sync.dma_start(bcols_s[:], bcols[:, :])
            brows_s = constp.tile([1, 3 * 128], BF16, tag="brows")
            nc.sync.dma_start(brows_s[:], brows[:, :])

            def blk(i):
                return wcat_s[:, i * 128:(i + 1) * 128]

            wWT_s = blk(W_AW)
            WhuT_s, WhrT_s, WhgT_s = blk(W_HU), blk(W_HR), blk(W_HG)
            WxuT_s, WxrT_s, WxgT_s = blk(W_XU), blk(W_XR), blk(W_XG)
            ln2wh_s, ln2wt_s, ident_s = blk(W_L2H), blk(W_L2T), blk(W_ID)
            nWhrT_s = blk(W_NHR)
            nWhgT_s = blk(W_NHG)
            ones_row_s = wcat_s[0:1, W_ONE * 128:(W_ONE + 1) * 128]
            ones_col_s = wcat_s[:, W_ONE * 128:W_ONE * 128 + 1]
            wb_s = bcols_s[:, 0:1]
            bu_s = bcols_s[:, 1:2]
            br_s = bcols_s[:, 2:3]
            bq_col_s = bcols_s[:, 3:4]
            ln2b_s = brows_s[:, 0:H]
            bg_row_s = brows_s[:, 128:128 + H]
            bq_row_s = brows_s[:, 256:256 + H]

            # ---- remaining input DMA slices, interleaved tgt/hist ----
            for d in range(1, NDMA):
                sl = slice(d * TSL, (d + 1) * TSL)
                nc.sync.dma_start(tgt_all[:, sl, :], tgtT[:, sl, :])
                nc.scalar.dma_start(hist_all[:, sl, :], histT[:, sl, :])

            # warm the ACT exp table while DMA streams
            dummy = attp.tile([1, 1], F32, tag="dummy")
            nc.vector.memset(dummy[:], 0.0)
            nc.scalar.activation(dummy[:], dummy[:], AF.Exp)

            attT = attp.tile([100, 2, BS], BF16, tag="attT")

            # ================= phase 1: attention =================
            with (
                tc.tile_pool(name="awps", bufs=2, space="PSUM") as awps,
                tc.tile_pool(name="lgps", bufs=1, space="PSUM") as lgps,
            ):
                logits_ps = lgps.tile([BS, t_steps], F32, tag="logits")
                for g in range(NPG):
                    t0 = g * PG
                    sl = slice(t0, t0 + PG)
                    awt = awps.tile([H, PG, BS], F32, tag="aw")
                    nc.tensor.matmul(
                        awt[:].rearrange("i t b -> i (t b)"),
                        wWT_s,
                        tgt_all[:, sl, :].rearrange("i t b -> i (t b)"),
                        start=True, stop=True)
                    # hoisted xg-projection for the scan (+ bq in the copy)
                    xqt = awps.tile([H, PG, BS], F32, tag="xq")
                    nc.tensor.matmul(
                        xqt[:].rearrange("i t b -> i (t b)"),
                        WxgT_s,
                        hist_all[:, sl, :].rearrange("i t b -> i (t b)"),
                        start=True, stop=True)
                    nc.scalar.activation(
                        xq_all[:, sl, :].rearrange("h t b -> h (t b)"),
                        xqt[:].rearrange("h t b -> h (t b)"),
                        AF.Identity, bias=bq_col_s)
                    # prod = (aw + W_b) * hist   (W_b fused per-partition)
                    prod = p1sb.tile([H, PG, BS], BF16, tag="prod")
                    nc.vector.scalar_tensor_tensor(
                        prod[:].rearrange("h t b -> h (t b)"),
                        awt[:].rearrange("h t b -> h (t b)"),
                        wb_s,
                        hist_all[:, sl, :].rearrange("h t b -> h (t b)"),
                        ALU.add, ALU.mult)
                    # logits[:, t] = ones^T . prod_t  (partition reduce on PE)
                    for j in range(PG):
                        nc.tensor.matmul(
                            logits_ps[:, t0 + j:t0 + j + 1],
                            prod[:, j, :],
                            ones_col_s,
                            start=True, stop=True)

                # ---- softmax over time (free dim) ----
                mx = attp.tile([BS, 1], F32, tag="mx")
                nc.vector.tensor_reduce(mx[:], logits_ps[:],
                                        mybir.AxisListType.X, ALU.max)
                negmx = attp.tile([BS, 1], F32, tag="negmx")
                nc.vector.tensor_scalar_mul(negmx[:], mx[:], -1.0)
                exps = attp.tile([BS, t_steps], F32, tag="exps")
                nc.scalar.activation(exps[:], logits_ps[:], AF.Exp,
                                     bias=negmx[:])
                # swap in the sigmoid/tanh table for the scan
                nc.scalar.activation(dummy[:], dummy[:], AF.Sigmoid)
                ssum = attp.tile([BS, 1], F32, tag="ssum")
                nc.vector.tensor_reduce(ssum[:], exps[:], mybir.AxisListType.X,
                                        ALU.add)
                rsum = attp.tile([BS, 1], F32, tag="rsum")
                nc.vector.reciprocal(rsum[:], ssum[:])
                att = attp.tile([BS, t_steps], BF16, tag="attn")
                nc.vector.tensor_scalar_mul(att[:], exps[:], rsum[:])
                # transpose att -> attT rows (PE transpose, two halves)
                for hf in range(2):
                    tps = awps.tile([100, BS], BF16, tag="aw")
                    nc.tensor.transpose(tps[:], att[:, hf * 100:(hf + 1) * 100],
                                        ident_s)
                    nc.vector.tensor_copy(attT[:, hf, :], tps[:])

            # ================= phase 2: the scan =================
            with (
                tc.tile_pool(name="rqps", bufs=2, space="PSUM") as rqps,
                tc.tile_pool(name="mps", bufs=2, space="PSUM") as mps,
                tc.tile_pool(name="ups", bufs=2, space="PSUM") as ups,
            ):
                h_t = statep.tile([H, BS], BF16, tag="h")
                nc.vector.memset(h_t[:], 0.0)

                rq_tiles = {}
                m_tiles = {}
                u_tiles = {}
                abc_tiles = {}
                row4_tiles = {}
                s1_prev = None
                t2_prev = None

                def row4_fill(c):
                    t0 = c * 4
                    row4 = scanp.tile([1, 4, BS], BF16, tag="arow")
                    nc.sync.dma_start(
                        row4[:], attT[t0 % 100:t0 % 100 + 4, t0 // 100, :])
                    row4_tiles[c] = row4

                def abc_fill(c):
                    row4 = row4_tiles.pop(c)
                    ab = scanp.tile([128, 4, BS], BF16, tag="absb")
                    nc.gpsimd.partition_broadcast(
                        ab[:].rearrange("p t b -> p (t b)"),
                        row4[:].rearrange("p t b -> p (t b)"))
                    abc_tiles[c] = ab

                def x_fill(t):
                    # x-side projections + bias outer-products for step t.
                    # rq bank [r | q]: readers sig_r then gpre (far apart).
                    # m bank: reader rm only.  u bank: reader sig_u only.
                    ht = hist_all[:, t, :]
                    rqt = rqps.tile([H, BS], F32, tag="rq")
                    mt = mps.tile([H, BS], F32, tag="m")
                    ut = ups.tile([H, BS], F32, tag="u")
                    nc.tensor.matmul(rqt[:], WxrT_s, ht,
                                     start=True, stop=False)
                    nc.tensor.matmul(mt[:], bg_row_s, ones_row_s[:, :BS],
                                     start=True, stop=False)
                    nc.tensor.matmul(ut[:], WxuT_s, ht,
                                     start=True, stop=False)
                    rq_tiles[t] = rqt
                    m_tiles[t] = mt
                    u_tiles[t] = ut

                def consume(t, h_cur):
                    nonlocal s1_prev, t2_prev
                    rqt = rq_tiles.pop(t)
                    mt = m_tiles.pop(t)
                    ut = u_tiles.pop(t)
                    ab = abc_tiles[t // 4]
                    if t > 0:
                        # r/g gates: Wh.h = -Wh.t2n + Wh.s1 (t2n-streams land
                        # during the previous tanh; only r_s1 is serial and
                        # it alone gates sig_r's bank)
                        nc.tensor.matmul(rqt[:], nWhrT_s, t2_prev[:],
                                         start=False, stop=False,
                                         skip_group_check=True)
                        nc.tensor.matmul(mt[:], nWhgT_s, t2_prev[:],
                                         start=False, stop=False,
                                         skip_group_check=True)
                        nc.tensor.matmul(rqt[:], WhrT_s, s1_prev[:],
                                         start=False, stop=True,
                                         skip_group_check=True)
                        nc.tensor.matmul(mt[:], WhgT_s, s1_prev[:],
                                         start=False, stop=True,
                                         skip_group_check=True)
                        nc.tensor.matmul(ut[:], WhuT_s, h_cur[:],
                                         start=False, stop=True,
                                         skip_group_check=True)
                    r = scanp.tile([H, BS], BF16, tag="r")
                    nc.scalar.activation(r[:], rqt[:], AF.Sigmoid,
                                         bias=br_s)
                    u = scanp.tile([H, BS], BF16, tag="u")
                    nc.scalar.activation(u[:], ut[:], AF.Sigmoid,
                                         bias=bu_s)
                    # rm = (mg + bg) * r     (bg pre-accumulated in PSUM)
                    rm = scanp.tile([H, BS], BF16, tag="rmv")
                    nc.vector.tensor_tensor(rm[:], mt[:], r[:], ALU.mult)
                    # gpre = (xq + bq) + rm  (xq+bq hoisted to phase 1,
                    # lives in SBUF bf16 -> DVE 2x mode)
                    gpre = scanp.tile([H, BS], BF16, tag="gpre")
                    bi_gpre = nc.vector.tensor_tensor(gpre[:], xq_all[:, t, :],
                                                      rm[:], ALU.add)
                    g_ = scanp.tile([H, BS], BF16, tag="g")
                    nc.scalar.activation(g_[:], gpre[:], AF.Tanh)
                    # v = a_t * u (DVE, behind gpre);
                    # t2n = (v - 1) * h = -(1-v)h in ONE fused op
                    v = scanp.tile([H, BS], BF16, tag="v")
                    bi_v = nc.vector.tensor_tensor(v[:], u[:], ab[:, t % 4, :],
                                                   ALU.mult)
                    add_dep_helper(bi_v.ins, bi_gpre.ins, sync=False,
                                   reason="scan: v after gpre (DVE order)")
                    t2 = scanp.tile([H, BS], BF16, tag="t2")
                    nc.vector.scalar_tensor_tensor(t2[:], v[:], 1.0, h_cur[:],
                                                   ALU.subtract, ALU.mult)
                    # s1 = g * v  (serial), then h' = s1 - t2n (off-path)
                    s1 = scanp.tile([H, BS], BF16, tag="s1")
                    nc.vector.tensor_tensor(s1[:], g_[:], v[:], ALU.mult)
                    h_new = statep.tile([H, BS], BF16, tag="h")
                    nc.vector.tensor_tensor(h_new[:], s1[:], t2[:],
                                            ALU.subtract)
                    s1_prev, t2_prev = s1, t2
                    return h_new

                row4_fill(0)
                row4_fill(1)
                abc_fill(0)
                for t in range(-XLA, t_steps):
                    if t >= 0 and t % 4 == 2:
                        c = t // 4
                        if c + 2 < NC4:
                            row4_fill(c + 2)
                        if c + 1 < NC4:
                            abc_fill(c + 1)
                    tf = t + XLA
                    if tf < t_steps:
                        x_fill(tf)
                    if t >= 0:
                        h_t = consume(t, h_t)

                # ---- phase 3: out = [h, targets[:,0]] @ ln2_w.T + ln2_b ----
                ops = rqps.tile([BS, H], F32, tag="rq")
                nc.tensor.matmul(ops[:], ones_row_s[:, :BS], ln2b_s,
                                 start=True, stop=False)
                nc.tensor.matmul(ops[:], h_t[:], ln2wh_s,
                                 start=False, stop=False, skip_group_check=True)
                nc.tensor.matmul(ops[:], tgt_all[:, 0, :], ln2wt_s,
                                 start=False, stop=True, skip_group_check=True)
                out_s = scanp.tile([BS, H], F32, tag="out_s")
                nc.vector.tensor_copy(out_s[:], ops[:])
                nc.sync.dma_start(out_d[:, :], out_s[:])

    nc.compile()
    return nc


def make_weight_feeds(inputs, t_steps=T):
    f32 = np.float32
    bf16 = ml_dtypes.bfloat16

    def tb(x):  # transpose to [in, out], fp32 -> bf16
        return np.ascontiguousarray(np.asarray(x, dtype=f32).T).astype(bf16)

    ln2_w = np.asarray(inputs["ln2_w"], f32)
    wblocks = [
        tb(inputs["W_w"]), tb(inputs["hu_w"]), tb(inputs["hr_w"]),
        tb(inputs["hg_w"]), tb(inputs["xu_w"]), tb(inputs["xr_w"]),
        tb(inputs["xg_w"]),
        np.ascontiguousarray(ln2_w[:, :H].T).astype(bf16),
        np.ascontiguousarray(ln2_w[:, H:].T).astype(bf16),
        np.eye(128, dtype=f32).astype(bf16),
        np.ones((128, 128), dtype=f32).astype(bf16),
        tb(-np.asarray(inputs["hr_w"], f32)),
        tb(-np.asarray(inputs["hg_w"], f32)),
    ]
    bcols = np.stack([
        np.asarray(inputs["W_b"], f32),
        np.asarray(inputs["xu_b"], f32) + np.asarray(inputs["hu_b"], f32),
        np.asarray(inputs["xr_b"], f32) + np.asarray(inputs["hr_b"], f32),
        np.asarray(inputs["xg_b"], f32),
    ], axis=1)
    brows = np.zeros((1, 3 * 128), f32)
    brows[0, :H] = np.asarray(inputs["ln2_b"], f32)
    brows[0, 128:128 + H] = np.asarray(inputs["hg_b"], f32)
    brows[0, 256:256 + H] = np.asarray(inputs["xg_b"], f32)
    return {
        "wcat": np.ascontiguousarray(np.concatenate(wblocks, axis=1)),
        "bcols": np.ascontiguousarray(bcols),
        "brows": brows.astype(bf16),
    }


def make_core_feeds(inputs, core, t_steps=T):
    bf16 = ml_dtypes.bfloat16
    sl = slice(core * BS, (core + 1) * BS)
    tgt = np.asarray(inputs["targets"])[sl, :t_steps]
    hist = np.asarray(inputs["history_states"])[sl, :t_steps]
    return {
        # [BS, T, F] -> [F, T, BS]
        "tgtT": np.ascontiguousarray(tgt.transpose(2, 1, 0)).astype(bf16),
        "histT": np.ascontiguousarray(hist.transpose(2, 1, 0)).astype(bf16),
    }


_nc_cache = {}


def _get_nc(t_steps=T):
    if t_steps not in _nc_cache:
        _nc_cache[t_steps] = build_nc(t_steps)
    return _nc_cache[t_steps]


def kernel(**inputs):
    nc = _get_nc(T)
    wf = make_weight_feeds(inputs)
    in_maps = [{**make_core_feeds(inputs, c), **wf} for c in range(NCORES)]
    res = run_bass_kernel_spmd(nc, in_maps, list(range(NCORES)))
    out = np.concatenate([res.results[c]["out"] for c in range(NCORES)], axis=0)
    return out.astype(np.float32)
